# revision 32
# baseline (speedup 1.0000x reference)
"""Corr1d-x-group cost-volume kernel for Trainium2 (8 NeuronCores, SPMD).

Data-parallel over batch N=8: core i processes batch i.

Per core (inputs [16, 256, 512] f32 each, output [108, 256, 512] f32):
  out[g*27+ch, h, w] = 0.25 * sum_c f1[g*4+c, h, w] * f2[g*4+c, h, w+ch-23]
with zero padding outside w in [0, 512).

Final design (282us; v3 baseline was 514us). The DVE is the binding
engine: 108 fp16 tensor_tensor products of [128, 4096] at ~2.2us each
(~238us); everything else is arranged so the DVE never waits:
  - Inputs host-pre-cast to fp16: halves load HBM traffic (16->8MB) and
    lets loads ride the HWDGE rings (sync/scalar) so the SWDGE queue
    carries only stores (no store backlog behind loads).
  - f2 is stored with 24 zero columns between the 8 hi-segments
    (segment stride 536). Shifted product reads then pull exact zeros
    outside the valid w-range instead of neighbor-row junk, so products
    are correct everywhere: no staging-buffer zeroing, no narrowed
    PSUM->SBUF copies, no per-shift edge memsets. f2 loads land compact
    (8KB runs) and a ScalarE copy fans them into the padded layout (a
    direct padded load shatters into 1KB descriptors).
  - Warmup product schedule: group-major over the first 2 shift
    positions so the DVE banks ~8 products while the loads stream in.
    (Capped by prodpool generation reuse: allocation #n reuses buffer
    #n-PROD_BUFS, whose consumer must already be emitted.)
  - c-reduction on TensorE: weight [128,32] = 0.25*(c-sum, hb-identity),
    4 groups packed per PSUM tile via tile_position col-tiling.
  - Staging is fp16; the store DMA casts fp16->f32 inline (SWDGE).
    Stores in 1MB halves (quarters on the final shift) per shift.
  - All 108 multiplies on DVE (GpSimd tensor_tensor contends with DVE
    for SBUF ports: tried and reverted).
  - ScalarE does PSUM->SBUF(fp16) copies, the f2 pad fan-out, and the
    odd-parity f2o copies (interleaved into early-shift slack).
"""

import os
import numpy as np

import concourse.bass as bass
import concourse.bacc as bacc
import concourse.mybir as mybir
import concourse.tile as tile
from concourse import bass_utils

N, C, H, W = 8, 16, 256, 512
G = 4
TOP_CH = 27
OUT_CH = G * TOP_CH  # 108
HB = 32   # h // 8 -> partition dim component
HI = 8    # h % 8  -> free dim component
FD = HI * W  # 4096
GAP = 24          # zero columns before each hi segment (covers s in [-23,3])
SEG = GAP + W     # 536
# 8 segments + trailing zeros; sized so the widest shifted [HI*SEG] slice
# (start GAP+3 even / GAP-1+3 odd) stays in bounds: >= GAP+3+HI*SEG = 4315.
F2W = HI * SEG + 32  # 4320

STG_BUFS = 4
PROD_BUFS = 7
F2C_BUFS = 2
# Even shifts (odd s_idx) first: they only need f2e, so the pipeline
# starts as soon as f1/f2e loads land; odd shifts run later, by which
# time the f2o odd-parity copies (SBUF->SBUF DMA) have long completed.
SHIFT_ORDER = list(range(1, TOP_CH, 2)) + list(range(0, TOP_CH, 2))

_CACHED = {}


def _reduction_weights() -> np.ndarray:
    # lhsT [K=(c,hb)=128, M=hb=32]: sums the 4 channels of a group and
    # applies the 1/sumelems scale.
    w = np.zeros((128, 32), np.float16)
    for c in range(G):
        for hb in range(HB):
            w[c * HB + hb, hb] = 0.25
    return w


def _build_program() -> bass.Bass:
    # Bacc (not raw Bass): its compile() splits multi-sem sync waits, which
    # TRN2 hardware limits to one per instruction.
    nc = bacc.Bacc(
        "TRN2",
        target_bir_lowering=False,
        debug=False,
        enable_asserts=False,
        num_devices=N,
    )
    f16 = mybir.dt.float16
    f32 = mybir.dt.float32

    # Inputs are pre-cast to fp16 on the host: halves the load HBM traffic
    # and removes the DMA dtype-cast, so loads can ride the HWDGE rings
    # (sync/scalar) instead of blocking the SWDGE store queue.
    l_in = nc.dram_tensor("l_in", [C, H, W], f16, kind="ExternalInput")
    r_in = nc.dram_tensor("r_in", [C, H, W], f16, kind="ExternalInput")
    w_red = nc.dram_tensor("w_red", [128, 32], f16, kind="ExternalInput")
    out = nc.dram_tensor("out", [OUT_CH, H, W], f32, kind="ExternalOutput")

    # DRAM views. h = hb*8 + hi.
    l_v = l_in.ap().rearrange(
        "(g c) (hb hi) w -> g (c hb) (hi w)", g=G, hb=HB, hi=HI
    )
    r_v = r_in.ap().rearrange(
        "(g c) (hb hi) w -> g (c hb) (hi w)", g=G, hb=HB, hi=HI
    )
    out_v = out.ap().rearrange(
        "(g s) (hb hi) w -> s g hb (hi w)", g=G, s=TOP_CH, hb=HB, hi=HI
    )

    with tile.TileContext(nc) as tc:
        with (
            tc.tile_pool(name="wpool", bufs=1) as wpool,
            tc.tile_pool(name="inpool", bufs=1) as inpool,
            tc.tile_pool(name="f2cpool", bufs=F2C_BUFS) as f2cpool,
            tc.tile_pool(name="prodpool", bufs=PROD_BUFS) as prodpool,
            tc.tile_pool(name="stgpool", bufs=STG_BUFS) as stgpool,
            tc.tile_pool(name="psumpool", bufs=4, space="PSUM") as psumpool,
        ):
            wt = wpool.tile([128, 32], f16)
            nc.sync.dma_start(wt[:], w_red[:])

            f1s, f2es, f2os = [], [], []
            for g in range(G):
                f1 = inpool.tile([128, FD], f16, tag=f"f1_{g}")
                nc.sync.dma_start(f1[:], l_v[g : g + 1])
                f1s.append(f1)
                f2e = inpool.tile([128, F2W], f16, tag=f"f2e_{g}")
                # Zero the inter-segment gaps + trailing columns (tiny DVE
                # memsets; run during the load phase while the DVE is idle).
                f2e_seg = f2e[:, : HI * SEG].rearrange(
                    "x (hi c) -> x hi c", hi=HI
                )
                nc.vector.memset(f2e_seg[:, :, 0:GAP], 0.0)
                nc.vector.memset(f2e[:, HI * SEG :], 0.0)
                # Compact load (one 8KB run/partition; a direct load into the
                # padded layout would shatter into 1KB descriptors), then a
                # ScalarE copy fans it out into the padded segments.
                f2c = f2cpool.tile([128, FD], f16, tag="f2c")
                nc.scalar.dma_start(f2c[:], r_v[g : g + 1])
                f2c3 = f2c[:].rearrange("x (hi w) -> x hi w", hi=HI)
                nc.scalar.copy(f2e_seg[:, :, GAP:SEG], f2c3[:])
                f2es.append(f2e)
                # Odd-parity copy tile: f2o[:, col] = f2e[:, col+1], so odd
                # shifts read 4B-aligned starts (keeps DVE 2x perf mode).
                # The copy itself is issued inside the shift loop (one per
                # early even shift) so it soaks up ScalarE slack instead of
                # blocking the first PSUM->SBUF copies or competing with the
                # input loads for SDMA engines.
                f2o = inpool.tile([128, F2W], f16, tag=f"f2o_{g}")
                nc.vector.memset(f2o[:, F2W - 2 :], 0.0)
                f2os.append(f2o)

            # Product schedule: group-major over the first 2 positions so the
            # DVE banks ~8 products while the (HBM-bound, ~45us) input loads
            # stream in group by group; shift-major afterwards. Capped at 2
            # positions: allocation #n reuses the buffer of #n-PROD_BUFS,
            # whose consumer block must already be emitted (3 positions
            # allocates 10 products before the first consumer -> deadlock).
            WARM = 2
            sched = [(p, g) for g in range(G) for p in range(WARM)]
            sched += [(p, g) for p in range(WARM, TOP_CH) for g in range(G)]
            prods = {}

            def emit_product(pos, g):
                s = SHIFT_ORDER[pos] - 23
                if s % 2 == 0:
                    src = f2es[g][:, GAP + s : GAP + s + HI * SEG]
                else:
                    src = f2os[g][:, GAP - 1 + s : GAP - 1 + s + HI * SEG]
                src3 = src.rearrange("x (hi c) -> x hi c", hi=HI)
                p = prodpool.tile([128, FD], f16, tag="prod")
                p3 = p[:].rearrange("x (hi w) -> x hi w", hi=HI)
                nc.vector.tensor_mul(p3[:], _f13(f1s[g]), src3[:, :, 0:W])
                prods[(pos, g)] = p

            for pos_g in sched:
                emit_product(*pos_g)
                pos = pos_g[0]
                if not all((pos, g) in prods for g in range(G)):
                    continue
                s_idx = SHIFT_ORDER[pos]
                stg = stgpool.tile([128, FD], f16, tag="stg")
                ps = [prods.pop((pos, g)) for g in range(G)]
                for j in range(4):  # hi pairs
                    pt = psumpool.tile([128, 1024], f32, tag="pt")
                    for k in range(2):
                        hi = 2 * j + k
                        for g in range(G):
                            nc.tensor.matmul(
                                pt[32 * g : 32 * g + 32,
                                   512 * k : 512 * (k + 1)],
                                wt[:],
                                ps[g][:, 512 * hi : 512 * (hi + 1)],
                                start=True,
                                stop=True,
                                tile_position=(0, 32 * g),
                            )
                    nc.scalar.copy(stg[:, 1024 * j : 1024 * (j + 1)], pt[:])
                    if pos < G and j == 1:
                        # Interleave one odd-parity copy into the ScalarE
                        # queue per early even shift: soaks up ACT slack well
                        # before the first odd shift needs f2o.
                        nc.scalar.copy(
                            f2os[pos][:, : F2W - 2], f2es[pos][:, 1 : F2W - 1]
                        )
                    if pos == TOP_CH - 1:
                        # Final shift: quarter-stores right after each copy
                        # so the trailing DMA drain is as short as possible.
                        nc.gpsimd.dma_start(
                            out_v[
                                s_idx : s_idx + 1, :, :,
                                1024 * j : 1024 * (j + 1),
                            ],
                            stg[:, 1024 * j : 1024 * (j + 1)],
                        )
                    elif j % 2 == 1:
                        # Store the finished half (SWDGE casts fp16 -> f32).
                        half = j // 2
                        nc.gpsimd.dma_start(
                            out_v[
                                s_idx : s_idx + 1, :, :,
                                2048 * half : 2048 * (half + 1),
                            ],
                            stg[:, 2048 * half : 2048 * (half + 1)],
                        )
    nc.compile()
    return nc


def _f13(f1):
    return f1[:].rearrange("x (hi w) -> x hi w", hi=HI)


def kernel(l_in: np.ndarray, r_in: np.ndarray) -> np.ndarray:
    assert l_in.shape == (N, C, H, W) and r_in.shape == (N, C, H, W)
    # Host-side fp16 pre-cast (products are computed in fp16 on-chip
    # regardless): halves load traffic and keeps loads off the store queue.
    l16 = np.ascontiguousarray(l_in.astype(np.float16))
    r16 = np.ascontiguousarray(r_in.astype(np.float16))

    if "nc" not in _CACHED:
        _CACHED["nc"] = _build_program()
    nc = _CACHED["nc"]

    w_np = _reduction_weights()
    in_maps = [
        {
            "l_in": np.ascontiguousarray(l16[i]),
            "r_in": np.ascontiguousarray(r16[i]),
            "w_red": w_np,
        }
        for i in range(N)
    ]
    trace = bool(int(os.environ.get("CORR_KERNEL_TRACE", "0")))
    kwargs = {}
    tdir = os.environ.get("CORR_KERNEL_TRACE_DIR")
    if trace and tdir:
        os.makedirs(tdir, exist_ok=True)
        kwargs["tmpdir"] = tdir
    res = bass_utils.run_bass_kernel_spmd(
        nc, in_maps, core_ids=list(range(N)), trace=trace, **kwargs
    )
    _CACHED["last_result"] = res
    return np.stack([res.results[i]["out"] for i in range(N)], axis=0)


# revision 34
# speedup vs baseline: 1.1664x; 1.1664x over previous
"""Corr1d-x-group cost-volume kernel for Trainium2 (8 NeuronCores, SPMD).

Data-parallel over batch N=8: core i processes batch i.

Per core (inputs [16, 256, 512] f32 each, output [108, 256, 512] f32):
  out[g*27+ch, h, w] = 0.25 * sum_c f1[g*4+c, h, w] * f2[g*4+c, h, w+ch-23]
with zero padding outside w in [0, 512).

Final design (282us; v3 baseline was 514us). The DVE is the binding
engine: 108 fp16 tensor_tensor products of [128, 4096] at ~2.2us each
(~238us); everything else is arranged so the DVE never waits:
  - Inputs host-pre-cast to fp16: halves load HBM traffic (16->8MB) and
    lets loads ride the HWDGE rings (sync/scalar) so the SWDGE queue
    carries only stores (no store backlog behind loads).
  - f2 is stored with 24 zero columns between the 8 hi-segments
    (segment stride 536). Shifted product reads then pull exact zeros
    outside the valid w-range instead of neighbor-row junk, so products
    are correct everywhere: no staging-buffer zeroing, no narrowed
    PSUM->SBUF copies, no per-shift edge memsets. f2 loads land compact
    (8KB runs) and a ScalarE copy fans them into the padded layout (a
    direct padded load shatters into 1KB descriptors).
  - Warmup product schedule: group-major over the first 2 shift
    positions so the DVE banks ~8 products while the loads stream in.
    (Capped by prodpool generation reuse: allocation #n reuses buffer
    #n-PROD_BUFS, whose consumer must already be emitted.)
  - c-reduction on TensorE: weight [128,32] = 0.25*(c-sum, hb-identity),
    4 groups packed per PSUM tile via tile_position col-tiling.
  - Staging is fp16; the store DMA casts fp16->f32 inline (SWDGE).
    Stores in 1MB halves (quarters on the final shift) per shift.
  - All 108 multiplies on DVE (GpSimd tensor_tensor contends with DVE
    for SBUF ports: tried and reverted).
  - ScalarE does PSUM->SBUF(fp16) copies, the f2 pad fan-out, and the
    odd-parity f2o copies (interleaved into early-shift slack).
"""

import os
import numpy as np

import concourse.bass as bass
import concourse.bacc as bacc
import concourse.mybir as mybir
import concourse.tile as tile
from concourse import bass_utils

N, C, H, W = 8, 16, 256, 512
G = 4
TOP_CH = 27
OUT_CH = G * TOP_CH  # 108
HB = 32   # h // 8 -> partition dim component
HI = 8    # h % 8  -> free dim component
FD = HI * W  # 4096
GAP = 24          # zero columns before each hi segment (covers s in [-23,3])
SEG = GAP + W     # 536
# 8 segments + trailing zeros; sized so the widest shifted [HI*SEG] slice
# (start GAP+3 even / GAP-1+3 odd) stays in bounds: >= GAP+3+HI*SEG = 4315.
F2W = HI * SEG + 32  # 4320

STG_BUFS = 4
PROD_BUFS = 7
F2C_BUFS = 2
# Even shifts (odd s_idx) first: they only need f2e, so the pipeline
# starts as soon as f1/f2e loads land; odd shifts run later, by which
# time the f2o odd-parity copies (SBUF->SBUF DMA) have long completed.
SHIFT_ORDER = list(range(1, TOP_CH, 2)) + list(range(0, TOP_CH, 2))

_CACHED = {}


def _reduction_weights() -> np.ndarray:
    # lhsT [K=(c,hb)=128, M=hb=32]: sums the 4 channels of a group and
    # applies the 1/sumelems scale.
    w = np.zeros((128, 32), np.float16)
    for c in range(G):
        for hb in range(HB):
            w[c * HB + hb, hb] = 0.25
    return w


def _build_program() -> bass.Bass:
    # Bacc (not raw Bass): its compile() splits multi-sem sync waits, which
    # TRN2 hardware limits to one per instruction.
    nc = bacc.Bacc(
        "TRN2",
        target_bir_lowering=False,
        debug=False,
        enable_asserts=False,
        num_devices=N,
    )
    f16 = mybir.dt.float16
    f32 = mybir.dt.float32

    # Inputs are pre-cast to fp16 on the host: halves the load HBM traffic
    # and removes the DMA dtype-cast, so loads can ride the HWDGE rings
    # (sync/scalar) instead of blocking the SWDGE store queue.
    l_in = nc.dram_tensor("l_in", [C, H, W], f16, kind="ExternalInput")
    r_in = nc.dram_tensor("r_in", [C, H, W], f16, kind="ExternalInput")
    w_red = nc.dram_tensor("w_red", [128, 32], f16, kind="ExternalInput")
    out = nc.dram_tensor("out", [OUT_CH, H, W], f32, kind="ExternalOutput")

    # DRAM views. h = hb*8 + hi.
    l_v = l_in.ap().rearrange(
        "(g c) (hb hi) w -> g (c hb) (hi w)", g=G, hb=HB, hi=HI
    )
    r_v = r_in.ap().rearrange(
        "(g c) (hb hi) w -> g (c hb) (hi w)", g=G, hb=HB, hi=HI
    )
    out_v = out.ap().rearrange(
        "(g s) (hb hi) w -> s g hb (hi w)", g=G, s=TOP_CH, hb=HB, hi=HI
    )

    with tile.TileContext(nc) as tc:
        with (
            tc.tile_pool(name="wpool", bufs=1) as wpool,
            tc.tile_pool(name="inpool", bufs=1) as inpool,
            tc.tile_pool(name="f2cpool", bufs=F2C_BUFS) as f2cpool,
            tc.tile_pool(name="prodpool", bufs=PROD_BUFS) as prodpool,
            tc.tile_pool(name="stgpool", bufs=STG_BUFS) as stgpool,
            tc.tile_pool(name="psumpool", bufs=4, space="PSUM") as psumpool,
        ):
            wt = wpool.tile([128, 32], f16)
            nc.sync.dma_start(wt[:], w_red[:])

            f1s, f2es, f2os = [], [], []
            for g in range(G):
                f1 = inpool.tile([128, FD], f16, tag=f"f1_{g}")
                nc.sync.dma_start(f1[:], l_v[g : g + 1])
                f1s.append(f1)
                f2e = inpool.tile([128, F2W], f16, tag=f"f2e_{g}")
                # Zero the inter-segment gaps + trailing columns (tiny DVE
                # memsets; run during the load phase while the DVE is idle).
                f2e_seg = f2e[:, : HI * SEG].rearrange(
                    "x (hi c) -> x hi c", hi=HI
                )
                nc.vector.memset(f2e_seg[:, :, 0:GAP], 0.0)
                nc.vector.memset(f2e[:, HI * SEG :], 0.0)
                # Compact load (one 8KB run/partition; a direct load into the
                # padded layout would shatter into 1KB descriptors), then a
                # ScalarE copy fans it out into the padded segments.
                f2c = f2cpool.tile([128, FD], f16, tag="f2c")
                nc.scalar.dma_start(f2c[:], r_v[g : g + 1])
                f2c3 = f2c[:].rearrange("x (hi w) -> x hi w", hi=HI)
                nc.scalar.copy(f2e_seg[:, :, GAP:SEG], f2c3[:])
                f2es.append(f2e)
                # Odd-parity copy tile: f2o[:, col] = f2e[:, col+1], so odd
                # shifts read 4B-aligned starts (keeps DVE 2x perf mode).
                # The copy itself is issued inside the shift loop (one per
                # early even shift) so it soaks up ScalarE slack instead of
                # blocking the first PSUM->SBUF copies or competing with the
                # input loads for SDMA engines.
                f2o = inpool.tile([128, F2W], f16, tag=f"f2o_{g}")
                nc.vector.memset(f2o[:, F2W - 2 :], 0.0)
                f2os.append(f2o)

            # Product schedule: group-major over the first 2 positions so the
            # DVE banks ~8 products while the (HBM-bound, ~45us) input loads
            # stream in group by group; shift-major afterwards. Capped at 2
            # positions: allocation #n reuses the buffer of #n-PROD_BUFS,
            # whose consumer block must already be emitted (3 positions
            # allocates 10 products before the first consumer -> deadlock).
            WARM = 2
            sched = [(p, g) for g in range(G) for p in range(WARM)]
            sched += [(p, g) for p in range(WARM, TOP_CH) for g in range(G)]
            prods = {}

            def emit_product(pos, g):
                s = SHIFT_ORDER[pos] - 23
                if s % 2 == 0:
                    src = f2es[g][:, GAP + s : GAP + s + HI * SEG]
                else:
                    src = f2os[g][:, GAP - 1 + s : GAP - 1 + s + HI * SEG]
                src3 = src.rearrange("x (hi c) -> x hi c", hi=HI)
                p = prodpool.tile([128, FD], f16, tag="prod")
                p3 = p[:].rearrange("x (hi w) -> x hi w", hi=HI)
                nc.vector.tensor_mul(p3[:], _f13(f1s[g]), src3[:, :, 0:W])
                prods[(pos, g)] = p

            for pos_g in sched:
                emit_product(*pos_g)
                pos = pos_g[0]
                if not all((pos, g) in prods for g in range(G)):
                    continue
                s_idx = SHIFT_ORDER[pos]
                stg = stgpool.tile([128, FD], f16, tag="stg")
                ps = [prods.pop((pos, g)) for g in range(G)]
                for j in range(4):  # hi pairs
                    pt = psumpool.tile([128, 1024], f32, tag="pt")
                    for k in range(2):
                        hi = 2 * j + k
                        for g in range(G):
                            nc.tensor.matmul(
                                pt[32 * g : 32 * g + 32,
                                   512 * k : 512 * (k + 1)],
                                wt[:],
                                ps[g][:, 512 * hi : 512 * (hi + 1)],
                                start=True,
                                stop=True,
                                tile_position=(0, 32 * g),
                            )
                    if pos == TOP_CH - 1 and j % 2 == 0:
                        # Final shift: the DVE has no multiplies left, so
                        # split the PSUM->SBUF copies across DVE and ScalarE
                        # to halve the serialized end-of-kernel copy chain.
                        nc.vector.tensor_copy(
                            stg[:, 1024 * j : 1024 * (j + 1)], pt[:]
                        )
                    else:
                        nc.scalar.copy(
                            stg[:, 1024 * j : 1024 * (j + 1)], pt[:]
                        )
                    if pos < G and j == 1:
                        # Interleave one odd-parity copy into the ScalarE
                        # queue per early even shift: soaks up ACT slack well
                        # before the first odd shift needs f2o.
                        nc.scalar.copy(
                            f2os[pos][:, : F2W - 2], f2es[pos][:, 1 : F2W - 1]
                        )
                    if pos >= TOP_CH - 2:
                        # Final shifts: quarter-stores right after each copy
                        # so the trailing DMA drain is as short as possible.
                        nc.gpsimd.dma_start(
                            out_v[
                                s_idx : s_idx + 1, :, :,
                                1024 * j : 1024 * (j + 1),
                            ],
                            stg[:, 1024 * j : 1024 * (j + 1)],
                        )
                    elif j % 2 == 1:
                        # Store the finished half (SWDGE casts fp16 -> f32).
                        half = j // 2
                        nc.gpsimd.dma_start(
                            out_v[
                                s_idx : s_idx + 1, :, :,
                                2048 * half : 2048 * (half + 1),
                            ],
                            stg[:, 2048 * half : 2048 * (half + 1)],
                        )
    nc.compile()
    return nc


def _f13(f1):
    return f1[:].rearrange("x (hi w) -> x hi w", hi=HI)


def kernel(l_in: np.ndarray, r_in: np.ndarray) -> np.ndarray:
    assert l_in.shape == (N, C, H, W) and r_in.shape == (N, C, H, W)
    # Host-side fp16 pre-cast (products are computed in fp16 on-chip
    # regardless): halves load traffic and keeps loads off the store queue.
    l16 = np.ascontiguousarray(l_in.astype(np.float16))
    r16 = np.ascontiguousarray(r_in.astype(np.float16))

    if "nc" not in _CACHED:
        _CACHED["nc"] = _build_program()
    nc = _CACHED["nc"]

    w_np = _reduction_weights()
    in_maps = [
        {
            "l_in": np.ascontiguousarray(l16[i]),
            "r_in": np.ascontiguousarray(r16[i]),
            "w_red": w_np,
        }
        for i in range(N)
    ]
    trace = bool(int(os.environ.get("CORR_KERNEL_TRACE", "0")))
    kwargs = {}
    tdir = os.environ.get("CORR_KERNEL_TRACE_DIR")
    if trace and tdir:
        os.makedirs(tdir, exist_ok=True)
        kwargs["tmpdir"] = tdir
    res = bass_utils.run_bass_kernel_spmd(
        nc, in_maps, core_ids=list(range(N)), trace=trace, **kwargs
    )
    _CACHED["last_result"] = res
    return np.stack([res.results[i]["out"] for i in range(N)], axis=0)
